# revision 2
# baseline (speedup 1.0000x reference)
"""Compressed MoE block on 8 Trainium2 NeuronCores.

Expert-parallel sharding: core e owns expert e. The router (tiny: T x H @
H x E) runs on host as part of dispatch; tokens are gathered per selected
expert (top-2), padded to a fixed capacity, and each core runs the full
factored FFN chain for its expert in token-transposed layout:

    g1T = Ug'(e).T @ xT          (Ug' = Ug @ Cg folded on host)
    gT  = Vg(e).T  @ g1T
    u1T = Uu'(e).T @ xT
    uT  = Vu(e).T  @ u1T
    aT  = silu(gT) * uT
    d1T = Ud'(e).T @ aT          (Ud' = Ud @ Cd)
    yT  = Vd(e).T  @ d1T

All matmuls run as float32r (full-rate PE path, ~1e-4 matmul precision).
Host scatters y back with the renormalized top-2 routing weights.
"""

import numpy as np

import concourse.bacc as bacc
import concourse.mybir as mybir
import concourse.tile as tile
from concourse.bass_utils import run_bass_kernel_spmd

F32 = mybir.dt.float32
F32R = mybir.dt.float32r

E = 8
KTOP = 2
H = 1024
FF = 2816
R = 256
KH = H // 128    # 8
KR = R // 128    # 2
KF = FF // 128   # 22
MH = H // 128    # 8

_BUILD_CACHE = {}
LAST_RESULT = None


def _build(C, nch):
    """Build the per-core bass program for capacity C split into nch chunks."""
    chunk = C // nch
    nc = bacc.Bacc()

    xtp = nc.declare_dram_parameter("xtp", [128, KH * C], F32R, isOutput=False)
    ugcp = nc.declare_dram_parameter("ugcp", [128, KH * R], F32R, isOutput=False)
    uucp = nc.declare_dram_parameter("uucp", [128, KH * R], F32R, isOutput=False)
    vgp = nc.declare_dram_parameter("vgp", [128, KR * FF], F32R, isOutput=False)
    vup = nc.declare_dram_parameter("vup", [128, KR * FF], F32R, isOutput=False)
    udcp = nc.declare_dram_parameter("udcp", [128, KF * R], F32R, isOutput=False)
    vdp = nc.declare_dram_parameter("vdp", [128, KR * H], F32R, isOutput=False)
    ytp = nc.declare_dram_parameter("ytp", [128, MH * C], F32, isOutput=True)

    with tile.TileContext(nc) as tc:
        with (
            tc.tile_pool(name="wsb", bufs=1) as wsb,
            tc.tile_pool(name="work", bufs=3) as work,
            tc.tile_pool(name="pmm", bufs=4, space="PSUM") as pmm,
            tc.tile_pool(name="pd1", bufs=4, space="PSUM") as pd1,
        ):
            xts = wsb.tile([128, KH * C], F32R, tag="xts")
            ugcs = wsb.tile([128, KH * R], F32R, tag="ugcs")
            uucs = wsb.tile([128, KH * R], F32R, tag="uucs")
            vgs = wsb.tile([128, KR * FF], F32R, tag="vgs")
            vus = wsb.tile([128, KR * FF], F32R, tag="vus")
            udcs = wsb.tile([128, KF * R], F32R, tag="udcs")
            vds = wsb.tile([128, KR * H], F32R, tag="vds")
            g1s = wsb.tile([128, KR * C], F32R, tag="g1s")
            u1s = wsb.tile([128, KR * C], F32R, tag="u1s")
            d1s = wsb.tile([128, KR * C], F32R, tag="d1s")

            # --- input DMAs, split for load/compute overlap, in consumption order
            half = (KH // 2) * C
            nc.sync.dma_start(xts[:, :half], xtp[:, :half])
            nc.sync.dma_start(xts[:, half:], xtp[:, half:])
            nc.sync.dma_start(ugcs[:], ugcp[:])
            nc.sync.dma_start(uucs[:], uucp[:])
            # vg/vu: pieces per (k, quarter of F)
            fq = [(0, 6), (6, 12), (12, 17), (17, 22)]
            for fa, fb in fq:
                for k in range(KR):
                    nc.sync.dma_start(
                        vgs[:, k * FF + fa * 128:k * FF + fb * 128],
                        vgp[:, k * FF + fa * 128:k * FF + fb * 128],
                    )
                    nc.sync.dma_start(
                        vus[:, k * FF + fa * 128:k * FF + fb * 128],
                        vup[:, k * FF + fa * 128:k * FF + fb * 128],
                    )
            for fa, fb in fq:
                nc.sync.dma_start(
                    udcs[:, fa * R:fb * R], udcp[:, fa * R:fb * R]
                )
            nc.sync.dma_start(vds[:], vdp[:])

            for n in range(nch):
                c0 = n * chunk
                # --- phase A: g1T/u1T [R, chunk] = Ug'/Uu'.T @ xT
                for src, dst in ((ugcs, g1s), (uucs, u1s)):
                    for m in range(KR):
                        ps = pmm.tile([128, chunk], F32, tag="mm")
                        for k in range(KH):
                            nc.tensor.matmul(
                                ps[:],
                                src[:, k * R + m * 128:k * R + (m + 1) * 128],
                                xts[:, k * C + c0:k * C + c0 + chunk],
                                start=(k == 0), stop=(k == KH - 1),
                            )
                        nc.vector.tensor_copy(
                            dst[:, m * C + c0:m * C + c0 + chunk], ps[:]
                        )

                # --- phase B: f-loop, fused silu*up and d1 accumulation
                d1p = [
                    pd1.tile([128, chunk], F32, tag="d1", name=f"d1p_{n}_{m}")
                    for m in range(KR)
                ]
                for f in range(KF):
                    gps = pmm.tile([128, chunk], F32, tag="mm")
                    for k in range(KR):
                        nc.tensor.matmul(
                            gps[:],
                            vgs[:, k * FF + f * 128:k * FF + (f + 1) * 128],
                            g1s[:, k * C + c0:k * C + c0 + chunk],
                            start=(k == 0), stop=(k == KR - 1),
                        )
                    ups = pmm.tile([128, chunk], F32, tag="mm")
                    for k in range(KR):
                        nc.tensor.matmul(
                            ups[:],
                            vus[:, k * FF + f * 128:k * FF + (f + 1) * 128],
                            u1s[:, k * C + c0:k * C + c0 + chunk],
                            start=(k == 0), stop=(k == KR - 1),
                        )
                    gsil = work.tile([128, chunk], F32, tag="gsil")
                    nc.scalar.activation(
                        gsil[:], gps[:], mybir.ActivationFunctionType.Silu
                    )
                    af = work.tile([128, chunk], F32R, tag="af")
                    nc.vector.tensor_mul(af[:], gsil[:], ups[:])
                    for m in range(KR):
                        nc.tensor.matmul(
                            d1p[m][:],
                            udcs[:, f * R + m * 128:f * R + (m + 1) * 128],
                            af[:],
                            start=(f == 0), stop=(f == KF - 1),
                        )
                for m in range(KR):
                    nc.vector.tensor_copy(
                        d1s[:, m * C + c0:m * C + c0 + chunk], d1p[m][:]
                    )

                # --- phase C: yT [H, chunk] = Vd.T @ d1T
                for m in range(MH):
                    yps = pmm.tile([128, chunk], F32, tag="mm")
                    for k in range(KR):
                        nc.tensor.matmul(
                            yps[:],
                            vds[:, k * H + m * 128:k * H + (m + 1) * 128],
                            d1s[:, k * C + c0:k * C + c0 + chunk],
                            start=(k == 0), stop=(k == KR - 1),
                        )
                    yts = work.tile([128, chunk], F32, tag="yts")
                    nc.vector.tensor_copy(yts[:], yps[:])
                    nc.sync.dma_start(
                        ytp[:, m * C + c0:m * C + c0 + chunk], yts[:]
                    )

    nc.finalize()
    return nc


def _pack(a, kt):
    """[kt*128, X] row-major -> [128, kt*X] partition-tiled."""
    x = a.shape[1]
    return np.ascontiguousarray(
        a.reshape(kt, 128, x).transpose(1, 0, 2).reshape(128, kt * x)
    )


def kernel(hidden_states, gate_w, Ug, Cg, Vg, Uu, Cu, Vu, Ud, Cd, Vd):
    global LAST_RESULT
    hidden_states = np.asarray(hidden_states, dtype=np.float32)
    gate_w = np.asarray(gate_w, dtype=np.float32)
    b, s, h = hidden_states.shape
    x = hidden_states.reshape(-1, h)
    T = x.shape[0]

    # --- router (host; part of dispatch)
    logits = (x @ gate_w).astype(np.float64)
    lmax = logits.max(axis=-1, keepdims=True)
    p = np.exp(logits - lmax)
    p /= p.sum(axis=-1, keepdims=True)
    i1 = np.argmax(p, axis=-1)
    p1 = p[np.arange(T), i1]
    p_masked = p.copy()
    p_masked[np.arange(T), i1] = -np.inf
    i2 = np.argmax(p_masked, axis=-1)
    p2 = p[np.arange(T), i2]
    w1 = (p1 / (p1 + p2)).astype(np.float32)
    w2 = (p2 / (p1 + p2)).astype(np.float32)

    idx_e = []
    wgt_e = []
    for e in range(E):
        sel1 = np.nonzero(i1 == e)[0]
        sel2 = np.nonzero(i2 == e)[0]
        ids = np.concatenate([sel1, sel2])
        ws = np.concatenate([w1[sel1], w2[sel2]])
        idx_e.append(ids)
        wgt_e.append(ws)

    max_n = max(len(ids) for ids in idx_e)
    nch = max(1, -(-max_n // 512))
    chunk = max(256, -(-max_n // (nch * 32)) * 32)
    C = nch * chunk

    key = (C, nch)
    if key not in _BUILD_CACHE:
        _BUILD_CACHE[key] = _build(C, nch)
    nc = _BUILD_CACHE[key]

    f32 = np.float32
    in_maps = []
    for e in range(E):
        ids = idx_e[e]
        xT = np.zeros((h, C), f32)
        xT[:, :len(ids)] = x[ids].T
        ugc = (Ug[e] @ Cg).astype(f32)
        uuc = (Uu[e] @ Cu).astype(f32)
        udc = (Ud[e] @ Cd).astype(f32)
        in_maps.append({
            "xtp": _pack(xT, KH),
            "ugcp": _pack(ugc, KH),
            "uucp": _pack(uuc, KH),
            "vgp": _pack(np.asarray(Vg[e], f32), KR),
            "vup": _pack(np.asarray(Vu[e], f32), KR),
            "udcp": _pack(udc, KF),
            "vdp": _pack(np.asarray(Vd[e], f32), KR),
        })

    res = run_bass_kernel_spmd(nc, in_maps, list(range(E)))
    LAST_RESULT = res

    out = np.zeros((T, h), f32)
    for e in range(E):
        ids = idx_e[e]
        ytp = res.results[e]["ytp"]
        yT = ytp.reshape(128, MH, C).transpose(1, 0, 2).reshape(h, C)
        out[ids] += wgt_e[e][:, None] * yT[:, :len(ids)].T
    return out.reshape(b, s, h)
